# revision 44
# baseline (speedup 1.0000x reference)
"""Causal single-head attention on 8 Trainium2 NeuronCores (Bass/Tile).

Problem: X [4, 2048, 1024] f32; W_q/W_k/W_v [1024, 1024] f32.
out[b] = softmax(mask((X[b] Wq)(X[b] Wk)^T / 32)) (X[b] Wv)

Sharding: 8 cores = 4 batches x 2 key-parity halves (partial softmax).
Core c = 2b + h owns batch b's key tiles {2j + h : j = 0..7} (128-row
tiles, interleaved so causal work per local tile j is j-independent across
cores). Each core computes unnormalized partial attention over its own
keys only and returns the partial numerator [2048, 1024] plus partial
softmax denominators; the host adds each pair's partials and divides.
Since exp needs no max-subtraction here (|scores/32| < ~4), partial
softmax combines exactly.

Merged QK weight: scores = (X Wq)(Xk Wk)^T = X (Wq Wk^T) Xk^T, so the
host folds W := Wk Wq^T once and the kernel computes scores as
(Xk W) X^T - the Q projection becomes a raw DMA of X^T.

fp8 DoubleRow matmuls: the PE runs fp8e4/e5 matmuls with
perf_mode=DoubleRow (two 128-row K-blocks per instruction) at 4x the
bf16 column rate. Precision is recovered with a split representation
a = a_h + a_l (hi: e4m3, residual: e5m2, effective ~7 mantissa bits):
a 2-term split (one operand split, other pure e4m3) halves PE time at
~1e-2 error; a 3-term split (both operands split, dropping the l*l
term) runs at 0.75x bf16 time with ~bf16 accuracy. Mix used here
(quadrature error ~1.4e-2 < 2e-2 gate, verified in numpy emulation):
  KW-proj  = wkq pure e4m3 (x64 host scale)  (x)  xk split   [2-term]
  scores   = KW pure e4m3 (psum copy)        (x)  xq split   [2-term]
  V-proj   = xk split  (x)  wv split (x32 host scale)        [3-term]
  AV       = w split (exp output)  (x)  V split              [3-term]
Power-of-2 scales (wkq x64, wv x32) keep the tiny merged weights out of
e4m3's subnormal floor; they fold into the exp scale (1/2048) and the
host's final divide - zero extra device ops.

All inputs are pre-packed on the host into the exact SBUF tile layouts
so every input DMA is whole-tile with >=1KB contiguous runs per
descriptor (the cost model halves DMA bandwidth below 512B runs).

Engine balance: PE is the bottleneck (~75us busy), so every non-matmul
op is spread across the three element-wise engines. Pool (GPSIMD)
cannot touch PSUM, so PSUM evacuations split between Act (exp, KW,
numerator ec0) and DVE (V staging, denominator, numerator ec1), and
Pool does all SBUF-only work (w/V hi+residual splits). Score chunks are
processed in 512-column pairs to halve per-op overhead; the first pair
of each key tile splits its w ops into 256-col halves so the
denominator/AV matmuls (which need only cols 0:256) start sooner. A
dummy Exp at build start preloads the Act function table off the
critical path.

All matmul contractions keep the contracted dim on partitions. DoubleRow
pairs adjacent 128-blocks: d-pairs for the projections, e-pairs for
scores, own-key-tile jj-pairs for AV (odd counts pair with a zeroed
128-col pad block at the end of the w tiles). Attention weights live
transposed (own keys on partitions) so they are directly usable as
matmul lhsT for numerator and denominator - no on-chip transposes.
"""

import sys

if "/opt/trn_rl_repo" not in sys.path:
    sys.path.insert(0, "/opt/trn_rl_repo")

import numpy as np

B, S, D = 4, 2048, 1024
HK = S // 2  # own key rows per core
P = 128
N_CORES = 8
# column offset of attention-weight block j inside the packed wt tile
WOFF = [0] * 9
for _j in range(8):
    WOFF[_j + 1] = WOFF[_j] + (16 - 2 * _j) * P
WTW = WOFF[8]  # 9216; cols [WTW, WTW+128) are a permanent zero pad block

_cache = {}


def _build_nc():
    from concourse import bacc
    import concourse.mybir as mybir
    import concourse.tile as tile

    fp32 = mybir.dt.float32
    bf16 = mybir.dt.bfloat16
    f8h = mybir.dt.float8e4  # e4m3: hi parts + pure operands
    f8l = mybir.dt.float8e5  # e5m2: residual parts
    Exp = mybir.ActivationFunctionType.Exp
    DR = mybir.MatmulPerfMode.DoubleRow

    nc = bacc.Bacc("TRN2", target_bir_lowering=False)

    # host-packed inputs: already in SBUF tile layout (partition-major)
    xk1h_d = nc.dram_tensor("xk1h", [P, 8, 512], f8h, kind="ExternalInput")
    xk1l_d = nc.dram_tensor("xk1l", [P, 8, 512], f8l, kind="ExternalInput")
    xk2h_d = nc.dram_tensor("xk2h", [P, 8, 512], f8h, kind="ExternalInput")
    xk2l_d = nc.dram_tensor("xk2l", [P, 8, 512], f8l, kind="ExternalInput")
    xqh_d = nc.dram_tensor("xqh", [P, 8, S], f8h, kind="ExternalInput")
    xql_d = nc.dram_tensor("xql", [P, 8, S], f8l, kind="ExternalInput")
    # merged weight in 8 per-e-block pieces: each KW output block's lhsT
    # lands as its own early whole-tile DMA, pacing the KW loop's needs
    wm_d = [
        nc.dram_tensor(f"wm{e}", [P, 8, P], f8h, kind="ExternalInput")
        for e in range(8)
    ]
    wvh_d = nc.dram_tensor("wvh", [P, 8, D], f8h, kind="ExternalInput")
    wvl_d = nc.dram_tensor("wvl", [P, 8, D], f8l, kind="ExternalInput")
    band_d = nc.dram_tensor("band", [P, 256], fp32, kind="ExternalInput")
    # partial numerator (x32) in bf16 (halves output DMA; the host pair-sum
    # and divide run in fp32) + per-q-tile denominator columns in fp32
    num_d = nc.dram_tensor("num", [S, D], bf16, kind="ExternalOutput")
    den_d = nc.dram_tensor("den", [P, 16], fp32, kind="ExternalOutput")

    with tile.TileContext(nc) as tc:
        with (
            tc.tile_pool(name="persist", bufs=1) as persist,
            tc.tile_pool(name="wtp", bufs=1) as wtp,
            tc.tile_pool(name="tmpp", bufs=3) as tmpp,
            tc.tile_pool(name="psS", bufs=4, space="PSUM") as psS,
        ):
            XQH = persist.tile([P, 8, S], f8h, tag="xqh")  # raw X^T hi
            XQL = persist.tile([P, 8, S], f8l, tag="xql")  # raw X^T residual
            KW = persist.tile([P, 8, HK], f8h, tag="kw")  # (Xk Wk Wq^T)^T x64
            VH = persist.tile([P, 8, D], f8h, tag="vh")  # Xk Wv x32 hi
            VL = persist.tile([P, 8, D], f8l, tag="vl")  # .. residual
            band = persist.tile([P, 256], fp32, tag="band")
            onesH = persist.tile([P, 1], f8h, tag="onesH")
            onesL = persist.tile([P, 1], f8l, tag="onesL")
            den_sb = persist.tile([P, 16], fp32, tag="den")  # col per q-tile
            dummy = persist.tile([P, 1], fp32, tag="dummy")
            nc.vector.memset(onesH[:], 1.0)
            nc.vector.memset(onesL[:], 1.0)
            # preload the Act engine's Exp table during the DMA-bound start
            nc.scalar.activation(dummy[:], onesH[:], Exp, scale=1.0)

            # ---- Phase A: projections (fp8 DoubleRow matmuls, fp32 PSUM) --
            with (
                tc.tile_pool(name="wts", bufs=1) as wp,
                tc.tile_pool(name="xts", bufs=1) as xtsp,
                tc.tile_pool(name="warm", bufs=1) as warmp,
                tc.tile_pool(name="psA", bufs=4, space="PSUM") as psA,
            ):
                Wm = [
                    wp.tile([P, 8, P], f8h, tag=f"wm{e}", name=f"wm{e}")
                    for e in range(8)
                ]
                WvH = wp.tile([P, 8, D], f8h, tag="wvh")
                WvL = wp.tile([P, 8, D], f8l, tag="wvl")

                # PE warm-up: matmul cost is halved only once the PE's busy
                # streak is ~3us old, and the streak survives sub-~0.8us idle
                # gaps. The input DMAs take a few us, so keep the streak
                # alive with tiny N=64 matmuls paced ~450ns apart by a Pool
                # copy-chain (ping-pong buffers force serialization; Pool
                # starts instantly and is otherwise idle in phase A).
                wl_ = warmp.tile([P, P], bf16, tag="warm_l")
                wa = warmp.tile([P, 256], bf16, tag="warm_a")
                wb = warmp.tile([P, 256], bf16, tag="warm_b")
                nc.gpsimd.memset(wl_[:], 0.0)
                nc.gpsimd.memset(wb[:], 0.0)
                ps_w = psA.tile([P, 512], fp32, tag="psA", name="warm")
                for i in range(12):
                    src, dst = (wb, wa) if i % 2 == 0 else (wa, wb)
                    nc.gpsimd.tensor_copy(dst[:], src[:])
                    nc.tensor.matmul(
                        ps_w[:, :64], wl_[:], dst[:, :64], start=True, stop=True
                    )

                xsH = [
                    xtsp.tile([P, 8, 512], f8h, tag="xh1", name="xh1"),
                    xtsp.tile([P, 8, 512], f8h, tag="xh2", name="xh2"),
                ]
                xsL = [
                    xtsp.tile([P, 8, 512], f8l, tag="xl1", name="xl1"),
                    xtsp.tile([P, 8, 512], f8l, tag="xl2", name="xl2"),
                ]
                nc.sync.dma_start(Wm[0][:], wm_d[0][:])
                nc.sync.dma_start(xsH[0][:], xk1h_d[:])
                nc.sync.dma_start(xsL[0][:], xk1l_d[:])
                for e in range(1, 8):
                    nc.sync.dma_start(Wm[e][:], wm_d[e][:])
                nc.sync.dma_start(xsH[1][:], xk2h_d[:])
                nc.sync.dma_start(xsL[1][:], xk2l_d[:])
                nc.sync.dma_start(WvH[:], wvh_d[:])
                nc.sync.dma_start(WvL[:], wvl_d[:])
                nc.sync.dma_start(band[:], band_d[:])

                def wm_ap(e, dp):
                    # lhsT [P, 2, 128] d-pair for output e-block
                    return Wm[e][:, 2 * dp : 2 * dp + 2, :]

                def xs_ap(sc, lo, t, dp, cols):
                    # rhs [P, 2, cols] d-pair of own-key chunk sc, col lo
                    xs = (xsH, xsL)[t]
                    return xs[sc][:, 2 * dp : 2 * dp + 2, lo : lo + cols]

                # KW[e, k-chunk] = (x64 merged weight)^T @ Xk^T chunk.
                # 2-term: pure-e4m3 Wm against xk hi + residual.
                for sc in range(2):
                    for e in range(8):
                        psum = psA.tile([P, 512], fp32, tag="psA")
                        for lo in (0, 256):
                            for t in range(2):
                                for dp in range(4):
                                    nc.tensor.matmul(
                                        psum[:, lo : lo + 256],
                                        wm_ap(e, dp),
                                        xs_ap(sc, lo, t, dp, 256),
                                        start=(t == 0 and dp == 0),
                                        stop=(t == 1 and dp == 3),
                                        perf_mode=DR,
                                    )
                        nc.scalar.activation(
                            KW[:, e, sc * 512 : (sc + 1) * 512],
                            psum[:],
                            mybir.ActivationFunctionType.Copy,
                        )
                # V[k-tile, e] = Xk chunk @ Wv (x32)  - 3-term split x split
                for sc in range(2):
                    for kti in range(4):
                        kt = 4 * sc + kti
                        for ec in range(2):
                            psum = psA.tile([P, 512], fp32, tag="psA")
                            for lo in (0, 256):
                                for t in range(3):
                                    xt, wv = ((0, WvH), (1, WvH), (0, WvL))[t]
                                    for dp in range(4):
                                        # lhsT = xk chunk cols [kti*128,+128)
                                        xl_ = (xsH, xsL)[xt][sc]
                                        lhs = xl_[
                                            :, 2 * dp : 2 * dp + 2, kti * P : (kti + 1) * P
                                        ]
                                        nc.tensor.matmul(
                                            psum[:, lo : lo + 256],
                                            lhs,
                                            wv[:, 2 * dp : 2 * dp + 2, ec * 512 + lo : ec * 512 + lo + 256],
                                            start=(t == 0 and dp == 0),
                                            stop=(t == 2 and dp == 3),
                                            perf_mode=DR,
                                        )
                            nc.scalar.activation(
                                VH[:, kt, ec * 512 : (ec + 1) * 512],
                                psum[:],
                                mybir.ActivationFunctionType.Copy,
                            )
                            nc.vector.tensor_sub(
                                VL[:, kt, ec * 512 : (ec + 1) * 512],
                                psum[:],
                                VH[:, kt, ec * 512 : (ec + 1) * 512],
                            )
                # scores consume raw X^T directly - no Q projection at all
                nc.sync.dma_start(XQH[:], xqh_d[:])
                nc.sync.dma_start(XQL[:], xql_d[:])

            # ---- Phase B: scores+exp interleaved with outputs, offset by
            # one key tile: at step j, tile j's scores run on the PE while
            # tile j-1's w splits (completed during j's scores) feed the
            # numerator/denominator - no exp/split chain ever stalls the PE,
            # and the PSUM evacuations stay spread over the whole phase.
            wtH = wtp.tile([P, WTW + P], f8h, tag="wth")
            wtL = wtp.tile([P, WTW + P], f8l, tag="wtl")
            nc.vector.memset(wtH[:, WTW:], 0.0)  # zero pad block for
            nc.vector.memset(wtL[:, WTW:], 0.0)  # odd jj-pair counts
            # 128-col-block views for strided jj-pair lhsT access
            wtHv = wtH[:].rearrange("p (n b) -> p n b", b=P)
            wtLv = wtL[:].rearrange("p (n b) -> p n b", b=P)

            def scores_for(j):
                # scores + exp for own key tile j; q-range [256j, 2048),
                # processed as 512-col chunk pairs to halve op overhead
                nch = 8 - j
                ch = 0
                while ch < nch:
                    wide = 512 if ch + 1 < nch else 256
                    q0 = 256 * j + 256 * ch
                    psum_s = psS.tile([P, 512], fp32, tag="psS")
                    for lo in (0, 256)[: wide // 256]:
                        for t, xq in enumerate((XQH, XQL)):
                            for ep in range(4):
                                nc.tensor.matmul(
                                    psum_s[:, lo : lo + 256],
                                    KW[:, 2 * ep : 2 * ep + 2, j * P : (j + 1) * P],
                                    xq[:, 2 * ep : 2 * ep + 2, q0 + lo : q0 + lo + 256],
                                    start=(t == 0 and ep == 0),
                                    stop=(t == 1 and ep == 3),
                                    perf_mode=DR,
                                )
                    # psum holds 2048*z (64 from wkq, 32 softmax scale)
                    tmp = tmpp.tile([P, 512], fp32, tag="tmp")
                    nc.scalar.activation(
                        tmp[:, :wide], psum_s[:, :wide], Exp, scale=1 / 2048.0
                    )
                    if ch == 0:
                        # diagonal block: causal 0/1 mask (parity in data)
                        nc.vector.tensor_mul(tmp[:, :256], tmp[:, :256], band[:])
                    wcol = WOFF[j] + 256 * ch
                    nc.gpsimd.tensor_copy(
                        wtH[:, wcol : wcol + wide], tmp[:, :wide]
                    )
                    nc.vector.tensor_sub(
                        wtL[:, wcol : wcol + wide],
                        tmp[:, :wide],
                        wtH[:, wcol : wcol + wide],
                    )
                    ch += wide // 256

            with (
                tc.tile_pool(name="outp", bufs=2) as outp,
                tc.tile_pool(name="psAV", bufs=3, space="PSUM") as psAV,
                tc.tile_pool(name="psD", bufs=1, space="PSUM") as psD,
            ):

                def emit(g):
                    nj = g // 2 + 1  # own key tiles jj with 2jj <= g
                    npair = (nj + 1) // 2
                    out_sb = outp.tile([P, D], bf16, tag="out")

                    def pair_ap(wv_, pp):
                        # lhsT [P, 2, P]: w blocks jj=2pp, 2pp+1 for this
                        # g; an odd tail pairs with the zero pad block.
                        jj = 2 * pp
                        c0 = WOFF[jj] // P + (g - 2 * jj)
                        if jj + 1 < nj:
                            c1 = WOFF[jj + 1] // P + (g - 2 * jj - 2)
                        else:
                            c1 = WTW // P
                        step = c1 - c0
                        return wv_[:, c0 :: step, :][:, 0:2, :]

                    psum_dn = psD.tile([P, 1], fp32, tag="psD")
                    for t, (wv_, on) in enumerate(
                        ((wtHv, onesH), (wtLv, onesL))
                    ):
                        for jj in range(nj):
                            c = WOFF[jj] // P + (g - 2 * jj)
                            nc.tensor.matmul(
                                psum_dn[:],
                                wv_[:, c],
                                on[:],
                                start=(t == 0 and jj == 0),
                                stop=(t == 1 and jj == nj - 1),
                            )
                    nc.scalar.activation(
                        den_sb[:, g : g + 1],
                        psum_dn[:],
                        mybir.ActivationFunctionType.Copy,
                    )
                    if g == 15:
                        # batched denominator: one tiny contiguous DMA,
                        # issued before the final AV so it is off the tail
                        nc.sync.dma_start(den_d[:], den_sb[:])
                    for ec in range(2):
                        psum_av = psAV.tile([P, 512], fp32, tag="psAV")
                        for lo in (0, 256):
                            terms = (
                                (wtHv, VH),
                                (wtLv, VH),
                                (wtHv, VL),
                            )
                            for t, (wv_, vv) in enumerate(terms):
                                for pp in range(npair):
                                    nc.tensor.matmul(
                                        psum_av[:, lo : lo + 256],
                                        pair_ap(wv_, pp),
                                        vv[:, 2 * pp : 2 * pp + 2, ec * 512 + lo : ec * 512 + lo + 256],
                                        start=(t == 0 and pp == 0),
                                        stop=(t == 2 and pp == npair - 1),
                                        perf_mode=DR,
                                    )
                        # e-half DMA right after its copy: the final
                        # copy->DMA chains overlap instead of serializing
                        if ec == 0:
                            nc.scalar.activation(
                                out_sb[:, :512],
                                psum_av[:],
                                mybir.ActivationFunctionType.Copy,
                            )
                            nc.sync.dma_start(
                                num_d[g * P : (g + 1) * P, :512],
                                out_sb[:, :512],
                            )
                        elif g < 15:
                            nc.vector.tensor_copy(
                                out_sb[:, 512:1024], psum_av[:]
                            )
                        else:
                            # final tile: split the evacuation across DVE and
                            # Act and DMA each half on its own ring - the
                            # drain-gating transfer shrinks to 256 columns
                            nc.vector.tensor_copy(
                                out_sb[:, 512:768], psum_av[:, :256]
                            )
                            nc.scalar.activation(
                                out_sb[:, 768:1024],
                                psum_av[:, 256:],
                                mybir.ActivationFunctionType.Copy,
                            )
                            nc.scalar.dma_start(
                                num_d[g * P : (g + 1) * P, 512:768],
                                out_sb[:, 512:768],
                            )
                    nc.sync.dma_start(
                        num_d[g * P : (g + 1) * P, 512 + 256 * (g == 15) :],
                        out_sb[:, 512 + 256 * (g == 15) :],
                    )

                for j in range(8):
                    scores_for(j)
                    if j >= 1:
                        emit(2 * (j - 1))
                        emit(2 * (j - 1) + 1)
                emit(14)
                emit(15)

    nc.compile()
    return nc


def _get_nc():
    if "nc" not in _cache:
        _cache["nc"] = _build_nc()
    return _cache["nc"]


def _parity_cols(h):
    return np.concatenate(
        [np.arange(P * (2 * j + h), P * (2 * j + h) + P) for j in range(8)]
    )


def _split8(a):
    """hi (e4m3) + residual (e5m2) split of a float32 array."""
    import ml_dtypes

    a = np.asarray(a, dtype=np.float32)
    h = a.astype(ml_dtypes.float8_e4m3)
    l = (a - h.astype(np.float32)).astype(ml_dtypes.float8_e5m2)
    return h, l


def _pack(a):
    """[D, n] row-major -> [P, 8, n] partition-major tile layout."""
    n = a.shape[1]
    return np.ascontiguousarray(a.reshape(8, P, n).transpose(1, 0, 2))


def kernel(X, W_q, W_k, W_v, _run_kwargs=None, _results_out=None):
    import ml_dtypes
    from concourse.bass_utils import run_bass_kernel_spmd

    f8 = ml_dtypes.float8_e4m3
    X = np.asarray(X, dtype=np.float32)
    W_q = np.asarray(W_q, dtype=np.float32)
    W_k = np.asarray(W_k, dtype=np.float32)
    # scores = (X Wq)(Xk Wk)^T = X (Wq Wk^T) Xk^T: fold the weight product.
    # x64 scale keeps the tiny merged weights out of e4m3 subnormals; it is
    # divided back out in the exp scale (1/2048).
    wm = _pack((64.0 * (W_k @ W_q.T)).astype(f8))
    # x32 on Wv likewise; divided back out in the host's final division.
    wvh, wvl = _split8(32.0 * np.asarray(W_v, dtype=np.float32))
    wvh, wvl = _pack(wvh), _pack(wvl)

    cols = [_parity_cols(0), _parity_cols(1)]
    per_batch = []
    for b in range(B):
        xqT = np.ascontiguousarray(X[b].T)
        xqh, xql = _split8(xqT)
        pb = []
        for h in range(2):
            xkh, xkl = xqh[:, cols[h]], xql[:, cols[h]]
            pb.append((_pack(xkh), _pack(xkl)))
        per_batch.append((_pack(xqh), _pack(xql), pb))
    bands = []
    for h in range(2):
        x = np.arange(256)[None, :]
        p = np.arange(P)[:, None]
        bands.append(np.ascontiguousarray((x >= p + P * h).astype(np.float32)))

    in_maps = []
    for c in range(N_CORES):
        b, h = divmod(c, 2)
        xqh, xql, pb = per_batch[b]
        xkh, xkl = pb[h]
        in_maps.append(
            {
                "xk1h": np.ascontiguousarray(xkh[:, :, 0:512]),
                "xk1l": np.ascontiguousarray(xkl[:, :, 0:512]),
                "xk2h": np.ascontiguousarray(xkh[:, :, 512:1024]),
                "xk2l": np.ascontiguousarray(xkl[:, :, 512:1024]),
                "xqh": xqh,
                "xql": xql,
                **{
                    f"wm{e}": np.ascontiguousarray(wm[:, :, e * P : (e + 1) * P])
                    for e in range(8)
                },
                "wvh": wvh,
                "wvl": wvl,
                "band": bands[h],
            }
        )

    nc = _get_nc()
    res = None
    for attempt in range(3):
        try:
            res = run_bass_kernel_spmd(
                nc, in_maps, core_ids=list(range(N_CORES)), **(_run_kwargs or {})
            )
            # materialize now: device failures surface lazily at fetch time,
            # and they must land inside this retry loop
            res.results = [
                {k: np.asarray(v) for k, v in r.items()} for r in res.results
            ]
            break
        except Exception:
            # transient device wedges (NRT_EXEC_UNIT_UNRECOVERABLE) usually
            # clear on retry; drop the poisoned PJRT client first
            if attempt == 2:
                raise
            print(f"kernel: device run failed (attempt {attempt}), retrying",
                  file=sys.stderr)
            import time

            try:
                import jax
                import jax.extend.backend

                jax.clear_caches()
                jax.extend.backend.clear_backends()
            except Exception:
                pass
            time.sleep(3)
    if _results_out is not None:
        _results_out.append(res)

    out = np.empty((B, S, D), dtype=np.float32)
    for b in range(B):
        r0, r1 = res.results[2 * b], res.results[2 * b + 1]
        num = r0["num"].astype(np.float32) + r1["num"].astype(np.float32)
        den = (r0["den"] + r1["den"]).T.reshape(S, 1)  # [p, g] -> row g*128+p
        # numerator carries the x32 Wv host scale; denominator does not
        out[b] = num / (32.0 * den)
    return out


# revision 45
# speedup vs baseline: 1.0234x; 1.0234x over previous
"""Causal single-head attention on 8 Trainium2 NeuronCores (Bass/Tile).

Problem: X [4, 2048, 1024] f32; W_q/W_k/W_v [1024, 1024] f32.
out[b] = softmax(mask((X[b] Wq)(X[b] Wk)^T / 32)) (X[b] Wv)

Sharding: 8 cores = 4 batches x 2 key-parity halves (partial softmax).
Core c = 2b + h owns batch b's key tiles {2j + h : j = 0..7} (128-row
tiles, interleaved so causal work per local tile j is j-independent across
cores). Each core computes unnormalized partial attention over its own
keys only and returns the partial numerator [2048, 1024] plus partial
softmax denominators; the host adds each pair's partials and divides.
Since exp needs no max-subtraction here (|scores/32| < ~4), partial
softmax combines exactly.

Merged QK weight: scores = (X Wq)(Xk Wk)^T = X (Wq Wk^T) Xk^T, so the
host folds W := Wk Wq^T once and the kernel computes scores as
(Xk W) X^T - the Q projection becomes a raw DMA of X^T.

fp8 DoubleRow matmuls: the PE runs fp8e4/e5 matmuls with
perf_mode=DoubleRow (two 128-row K-blocks per instruction) at 4x the
bf16 column rate. Precision is recovered with a split representation
a = a_h + a_l (hi: e4m3, residual: e5m2, effective ~7 mantissa bits):
a 2-term split (one operand split, other pure e4m3) halves PE time at
~1e-2 error; a 3-term split (both operands split, dropping the l*l
term) runs at 0.75x bf16 time with ~bf16 accuracy. Mix used here
(quadrature error ~1.4e-2 < 2e-2 gate, verified in numpy emulation):
  KW-proj  = wkq pure e4m3 (x64 host scale)  (x)  xk split   [2-term]
  scores   = KW pure e4m3 (psum copy)        (x)  xq split   [2-term]
  V-proj   = xk split  (x)  wv split (x32 host scale)        [3-term]
  AV       = w split (exp output)  (x)  V split              [3-term]
Power-of-2 scales (wkq x64, wv x32) keep the tiny merged weights out of
e4m3's subnormal floor; they fold into the exp scale (1/2048) and the
host's final divide - zero extra device ops.

All inputs are pre-packed on the host into the exact SBUF tile layouts
so every input DMA is whole-tile with >=1KB contiguous runs per
descriptor (the cost model halves DMA bandwidth below 512B runs).

Engine balance: PE is the bottleneck (~75us busy), so every non-matmul
op is spread across the three element-wise engines. Pool (GPSIMD)
cannot touch PSUM, so PSUM evacuations split between Act (exp, KW,
numerator ec0) and DVE (V staging, denominator, numerator ec1), and
Pool does all SBUF-only work (w/V hi+residual splits). Score chunks are
processed in 512-column pairs to halve per-op overhead; the first pair
of each key tile splits its w ops into 256-col halves so the
denominator/AV matmuls (which need only cols 0:256) start sooner. A
dummy Exp at build start preloads the Act function table off the
critical path.

All matmul contractions keep the contracted dim on partitions. DoubleRow
pairs adjacent 128-blocks: d-pairs for the projections, e-pairs for
scores, own-key-tile jj-pairs for AV (odd counts pair with a zeroed
128-col pad block at the end of the w tiles). Attention weights live
transposed (own keys on partitions) so they are directly usable as
matmul lhsT for numerator and denominator - no on-chip transposes.
"""

import sys

if "/opt/trn_rl_repo" not in sys.path:
    sys.path.insert(0, "/opt/trn_rl_repo")

import numpy as np

B, S, D = 4, 2048, 1024
HK = S // 2  # own key rows per core
P = 128
N_CORES = 8
# column offset of attention-weight block j inside the packed wt tile
WOFF = [0] * 9
for _j in range(8):
    WOFF[_j + 1] = WOFF[_j] + (16 - 2 * _j) * P
WTW = WOFF[8]  # 9216; cols [WTW, WTW+128) are a permanent zero pad block

_cache = {}


def _build_nc():
    from concourse import bacc
    import concourse.mybir as mybir
    import concourse.tile as tile

    fp32 = mybir.dt.float32
    bf16 = mybir.dt.bfloat16
    f8h = mybir.dt.float8e4  # e4m3: hi parts + pure operands
    f8l = mybir.dt.float8e5  # e5m2: residual parts
    Exp = mybir.ActivationFunctionType.Exp
    DR = mybir.MatmulPerfMode.DoubleRow

    nc = bacc.Bacc("TRN2", target_bir_lowering=False)

    # host-packed inputs: already in SBUF tile layout (partition-major)
    xk1h_d = nc.dram_tensor("xk1h", [P, 8, 512], f8h, kind="ExternalInput")
    xk1l_d = nc.dram_tensor("xk1l", [P, 8, 512], f8l, kind="ExternalInput")
    xk2h_d = nc.dram_tensor("xk2h", [P, 8, 512], f8h, kind="ExternalInput")
    xk2l_d = nc.dram_tensor("xk2l", [P, 8, 512], f8l, kind="ExternalInput")
    xqh_d = nc.dram_tensor("xqh", [P, 8, S], f8h, kind="ExternalInput")
    xql_d = nc.dram_tensor("xql", [P, 8, S], f8l, kind="ExternalInput")
    # merged weight in 8 per-e-block pieces: each KW output block's lhsT
    # lands as its own early whole-tile DMA, pacing the KW loop's needs
    wm_d = [
        nc.dram_tensor(f"wm{e}", [P, 8, P], f8h, kind="ExternalInput")
        for e in range(8)
    ]
    wvh_d = nc.dram_tensor("wvh", [P, 8, D], f8h, kind="ExternalInput")
    wvl_d = nc.dram_tensor("wvl", [P, 8, D], f8l, kind="ExternalInput")
    band_d = nc.dram_tensor("band", [P, 256], fp32, kind="ExternalInput")
    # partial numerator (x32) in bf16 (halves output DMA; the host pair-sum
    # and divide run in fp32) + per-q-tile denominator columns in fp32
    num_d = nc.dram_tensor("num", [S, D], bf16, kind="ExternalOutput")
    den_d = nc.dram_tensor("den", [P, 16], fp32, kind="ExternalOutput")

    with tile.TileContext(nc) as tc:
        with (
            tc.tile_pool(name="persist", bufs=1) as persist,
            tc.tile_pool(name="wtp", bufs=1) as wtp,
            tc.tile_pool(name="tmpp", bufs=6) as tmpp,
            tc.tile_pool(name="psS", bufs=4, space="PSUM") as psS,
        ):
            XQH = persist.tile([P, 8, S], f8h, tag="xqh")  # raw X^T hi
            XQL = persist.tile([P, 8, S], f8l, tag="xql")  # raw X^T residual
            KW = persist.tile([P, 8, HK], f8h, tag="kw")  # (Xk Wk Wq^T)^T x64
            VH = persist.tile([P, 8, D], f8h, tag="vh")  # Xk Wv x32 hi
            VL = persist.tile([P, 8, D], f8l, tag="vl")  # .. residual
            band = persist.tile([P, 256], fp32, tag="band")
            onesH = persist.tile([P, 1], f8h, tag="onesH")
            onesL = persist.tile([P, 1], f8l, tag="onesL")
            den_sb = persist.tile([P, 16], fp32, tag="den")  # col per q-tile
            dummy = persist.tile([P, 1], fp32, tag="dummy")
            nc.vector.memset(onesH[:], 1.0)
            nc.vector.memset(onesL[:], 1.0)
            # preload the Act engine's Exp table during the DMA-bound start
            nc.scalar.activation(dummy[:], onesH[:], Exp, scale=1.0)

            # ---- Phase A: projections (fp8 DoubleRow matmuls, fp32 PSUM) --
            with (
                tc.tile_pool(name="wts", bufs=1) as wp,
                tc.tile_pool(name="xts", bufs=1) as xtsp,
                tc.tile_pool(name="warm", bufs=1) as warmp,
                tc.tile_pool(name="psA", bufs=4, space="PSUM") as psA,
            ):
                Wm = [
                    wp.tile([P, 8, P], f8h, tag=f"wm{e}", name=f"wm{e}")
                    for e in range(8)
                ]
                WvH = wp.tile([P, 8, D], f8h, tag="wvh")
                WvL = wp.tile([P, 8, D], f8l, tag="wvl")

                # PE warm-up: matmul cost is halved only once the PE's busy
                # streak is ~3us old, and the streak survives sub-~0.8us idle
                # gaps. The input DMAs take a few us, so keep the streak
                # alive with tiny N=64 matmuls paced ~450ns apart by a Pool
                # copy-chain (ping-pong buffers force serialization; Pool
                # starts instantly and is otherwise idle in phase A).
                wl_ = warmp.tile([P, P], bf16, tag="warm_l")
                wa = warmp.tile([P, 256], bf16, tag="warm_a")
                wb = warmp.tile([P, 256], bf16, tag="warm_b")
                nc.gpsimd.memset(wl_[:], 0.0)
                nc.gpsimd.memset(wb[:], 0.0)
                ps_w = psA.tile([P, 512], fp32, tag="psA", name="warm")
                for i in range(12):
                    src, dst = (wb, wa) if i % 2 == 0 else (wa, wb)
                    nc.gpsimd.tensor_copy(dst[:], src[:])
                    nc.tensor.matmul(
                        ps_w[:, :64], wl_[:], dst[:, :64], start=True, stop=True
                    )

                xsH = [
                    xtsp.tile([P, 8, 512], f8h, tag="xh1", name="xh1"),
                    xtsp.tile([P, 8, 512], f8h, tag="xh2", name="xh2"),
                ]
                xsL = [
                    xtsp.tile([P, 8, 512], f8l, tag="xl1", name="xl1"),
                    xtsp.tile([P, 8, 512], f8l, tag="xl2", name="xl2"),
                ]
                nc.sync.dma_start(Wm[0][:], wm_d[0][:])
                nc.sync.dma_start(xsH[0][:], xk1h_d[:])
                nc.sync.dma_start(xsL[0][:], xk1l_d[:])
                for e in range(1, 8):
                    nc.sync.dma_start(Wm[e][:], wm_d[e][:])
                nc.sync.dma_start(xsH[1][:], xk2h_d[:])
                nc.sync.dma_start(xsL[1][:], xk2l_d[:])
                nc.sync.dma_start(WvH[:], wvh_d[:])
                nc.sync.dma_start(WvL[:], wvl_d[:])
                nc.sync.dma_start(band[:], band_d[:])

                def wm_ap(e, dp):
                    # lhsT [P, 2, 128] d-pair for output e-block
                    return Wm[e][:, 2 * dp : 2 * dp + 2, :]

                def xs_ap(sc, lo, t, dp, cols):
                    # rhs [P, 2, cols] d-pair of own-key chunk sc, col lo
                    xs = (xsH, xsL)[t]
                    return xs[sc][:, 2 * dp : 2 * dp + 2, lo : lo + cols]

                # KW[e, k-chunk] = (x64 merged weight)^T @ Xk^T chunk.
                # 2-term: pure-e4m3 Wm against xk hi + residual.
                for sc in range(2):
                    for e in range(8):
                        psum = psA.tile([P, 512], fp32, tag="psA")
                        for lo in (0, 256):
                            for t in range(2):
                                for dp in range(4):
                                    nc.tensor.matmul(
                                        psum[:, lo : lo + 256],
                                        wm_ap(e, dp),
                                        xs_ap(sc, lo, t, dp, 256),
                                        start=(t == 0 and dp == 0),
                                        stop=(t == 1 and dp == 3),
                                        perf_mode=DR,
                                    )
                        nc.scalar.activation(
                            KW[:, e, sc * 512 : (sc + 1) * 512],
                            psum[:],
                            mybir.ActivationFunctionType.Copy,
                        )
                # V[k-tile, e] = Xk chunk @ Wv (x32)  - 3-term split x split
                for sc in range(2):
                    for kti in range(4):
                        kt = 4 * sc + kti
                        for ec in range(2):
                            psum = psA.tile([P, 512], fp32, tag="psA")
                            for lo in (0, 256):
                                for t in range(3):
                                    xt, wv = ((0, WvH), (1, WvH), (0, WvL))[t]
                                    for dp in range(4):
                                        # lhsT = xk chunk cols [kti*128,+128)
                                        xl_ = (xsH, xsL)[xt][sc]
                                        lhs = xl_[
                                            :, 2 * dp : 2 * dp + 2, kti * P : (kti + 1) * P
                                        ]
                                        nc.tensor.matmul(
                                            psum[:, lo : lo + 256],
                                            lhs,
                                            wv[:, 2 * dp : 2 * dp + 2, ec * 512 + lo : ec * 512 + lo + 256],
                                            start=(t == 0 and dp == 0),
                                            stop=(t == 2 and dp == 3),
                                            perf_mode=DR,
                                        )
                            nc.scalar.activation(
                                VH[:, kt, ec * 512 : (ec + 1) * 512],
                                psum[:],
                                mybir.ActivationFunctionType.Copy,
                            )
                            nc.vector.tensor_sub(
                                VL[:, kt, ec * 512 : (ec + 1) * 512],
                                psum[:],
                                VH[:, kt, ec * 512 : (ec + 1) * 512],
                            )
                # scores consume raw X^T directly - no Q projection at all
                nc.sync.dma_start(XQH[:], xqh_d[:])
                nc.sync.dma_start(XQL[:], xql_d[:])

            # ---- Phase B: scores+exp interleaved with outputs, offset by
            # one key tile: at step j, tile j's scores run on the PE while
            # tile j-1's w splits (completed during j's scores) feed the
            # numerator/denominator - no exp/split chain ever stalls the PE,
            # and the PSUM evacuations stay spread over the whole phase.
            wtH = wtp.tile([P, WTW + P], f8h, tag="wth")
            wtL = wtp.tile([P, WTW + P], f8l, tag="wtl")
            nc.vector.memset(wtH[:, WTW:], 0.0)  # zero pad block for
            nc.vector.memset(wtL[:, WTW:], 0.0)  # odd jj-pair counts
            # 128-col-block views for strided jj-pair lhsT access
            wtHv = wtH[:].rearrange("p (n b) -> p n b", b=P)
            wtLv = wtL[:].rearrange("p (n b) -> p n b", b=P)

            def scores_for(j):
                # scores + exp for own key tile j; q-range [256j, 2048),
                # processed as 512-col chunk pairs to halve op overhead
                nch = 8 - j
                ch = 0
                while ch < nch:
                    wide = 512 if ch + 1 < nch else 256
                    q0 = 256 * j + 256 * ch
                    psum_s = psS.tile([P, 512], fp32, tag="psS")
                    for lo in (0, 256)[: wide // 256]:
                        for t, xq in enumerate((XQH, XQL)):
                            for ep in range(4):
                                nc.tensor.matmul(
                                    psum_s[:, lo : lo + 256],
                                    KW[:, 2 * ep : 2 * ep + 2, j * P : (j + 1) * P],
                                    xq[:, 2 * ep : 2 * ep + 2, q0 + lo : q0 + lo + 256],
                                    start=(t == 0 and ep == 0),
                                    stop=(t == 1 and ep == 3),
                                    perf_mode=DR,
                                )
                    # psum holds 2048*z (64 from wkq, 32 softmax scale)
                    tmp = tmpp.tile([P, 512], fp32, tag="tmp")
                    nc.scalar.activation(
                        tmp[:, :wide], psum_s[:, :wide], Exp, scale=1 / 2048.0
                    )
                    if ch == 0:
                        # diagonal block: causal 0/1 mask (parity in data)
                        nc.vector.tensor_mul(tmp[:, :256], tmp[:, :256], band[:])
                    wcol = WOFF[j] + 256 * ch
                    nc.gpsimd.tensor_copy(
                        wtH[:, wcol : wcol + wide], tmp[:, :wide]
                    )
                    nc.vector.tensor_sub(
                        wtL[:, wcol : wcol + wide],
                        tmp[:, :wide],
                        wtH[:, wcol : wcol + wide],
                    )
                    ch += wide // 256

            with (
                tc.tile_pool(name="outp", bufs=4) as outp,
                tc.tile_pool(name="psAV", bufs=3, space="PSUM") as psAV,
                tc.tile_pool(name="psD", bufs=1, space="PSUM") as psD,
            ):

                def emit(g):
                    nj = g // 2 + 1  # own key tiles jj with 2jj <= g
                    npair = (nj + 1) // 2
                    out_sb = outp.tile([P, D], bf16, tag="out")

                    def pair_ap(wv_, pp):
                        # lhsT [P, 2, P]: w blocks jj=2pp, 2pp+1 for this
                        # g; an odd tail pairs with the zero pad block.
                        jj = 2 * pp
                        c0 = WOFF[jj] // P + (g - 2 * jj)
                        if jj + 1 < nj:
                            c1 = WOFF[jj + 1] // P + (g - 2 * jj - 2)
                        else:
                            c1 = WTW // P
                        step = c1 - c0
                        return wv_[:, c0 :: step, :][:, 0:2, :]

                    psum_dn = psD.tile([P, 1], fp32, tag="psD")
                    for t, (wv_, on) in enumerate(
                        ((wtHv, onesH), (wtLv, onesL))
                    ):
                        for jj in range(nj):
                            c = WOFF[jj] // P + (g - 2 * jj)
                            nc.tensor.matmul(
                                psum_dn[:],
                                wv_[:, c],
                                on[:],
                                start=(t == 0 and jj == 0),
                                stop=(t == 1 and jj == nj - 1),
                            )
                    nc.scalar.activation(
                        den_sb[:, g : g + 1],
                        psum_dn[:],
                        mybir.ActivationFunctionType.Copy,
                    )
                    if g == 15:
                        # batched denominator: one tiny contiguous DMA,
                        # issued before the final AV so it is off the tail
                        nc.sync.dma_start(den_d[:], den_sb[:])
                    for ec in range(2):
                        psum_av = psAV.tile([P, 512], fp32, tag="psAV")
                        for lo in (0, 256):
                            terms = (
                                (wtHv, VH),
                                (wtLv, VH),
                                (wtHv, VL),
                            )
                            for t, (wv_, vv) in enumerate(terms):
                                for pp in range(npair):
                                    nc.tensor.matmul(
                                        psum_av[:, lo : lo + 256],
                                        pair_ap(wv_, pp),
                                        vv[:, 2 * pp : 2 * pp + 2, ec * 512 + lo : ec * 512 + lo + 256],
                                        start=(t == 0 and pp == 0),
                                        stop=(t == 2 and pp == npair - 1),
                                        perf_mode=DR,
                                    )
                        # e-half DMA right after its copy: the final
                        # copy->DMA chains overlap instead of serializing
                        if ec == 0:
                            nc.scalar.activation(
                                out_sb[:, :512],
                                psum_av[:],
                                mybir.ActivationFunctionType.Copy,
                            )
                            nc.sync.dma_start(
                                num_d[g * P : (g + 1) * P, :512],
                                out_sb[:, :512],
                            )
                        elif g < 15:
                            nc.vector.tensor_copy(
                                out_sb[:, 512:1024], psum_av[:]
                            )
                        else:
                            # final tile: split the evacuation across DVE and
                            # Act and DMA each half on its own ring - the
                            # drain-gating transfer shrinks to 256 columns
                            nc.vector.tensor_copy(
                                out_sb[:, 512:768], psum_av[:, :256]
                            )
                            nc.scalar.activation(
                                out_sb[:, 768:1024],
                                psum_av[:, 256:],
                                mybir.ActivationFunctionType.Copy,
                            )
                            nc.scalar.dma_start(
                                num_d[g * P : (g + 1) * P, 512:768],
                                out_sb[:, 512:768],
                            )
                    nc.sync.dma_start(
                        num_d[g * P : (g + 1) * P, 512 + 256 * (g == 15) :],
                        out_sb[:, 512 + 256 * (g == 15) :],
                    )

                for j in range(8):
                    scores_for(j)
                    if j >= 1:
                        emit(2 * (j - 1))
                        emit(2 * (j - 1) + 1)
                emit(14)
                emit(15)

    nc.compile()
    return nc


def _get_nc():
    if "nc" not in _cache:
        _cache["nc"] = _build_nc()
    return _cache["nc"]


def _parity_cols(h):
    return np.concatenate(
        [np.arange(P * (2 * j + h), P * (2 * j + h) + P) for j in range(8)]
    )


def _split8(a):
    """hi (e4m3) + residual (e5m2) split of a float32 array."""
    import ml_dtypes

    a = np.asarray(a, dtype=np.float32)
    h = a.astype(ml_dtypes.float8_e4m3)
    l = (a - h.astype(np.float32)).astype(ml_dtypes.float8_e5m2)
    return h, l


def _pack(a):
    """[D, n] row-major -> [P, 8, n] partition-major tile layout."""
    n = a.shape[1]
    return np.ascontiguousarray(a.reshape(8, P, n).transpose(1, 0, 2))


def kernel(X, W_q, W_k, W_v, _run_kwargs=None, _results_out=None):
    import ml_dtypes
    from concourse.bass_utils import run_bass_kernel_spmd

    f8 = ml_dtypes.float8_e4m3
    X = np.asarray(X, dtype=np.float32)
    W_q = np.asarray(W_q, dtype=np.float32)
    W_k = np.asarray(W_k, dtype=np.float32)
    # scores = (X Wq)(Xk Wk)^T = X (Wq Wk^T) Xk^T: fold the weight product.
    # x64 scale keeps the tiny merged weights out of e4m3 subnormals; it is
    # divided back out in the exp scale (1/2048).
    wm = _pack((64.0 * (W_k @ W_q.T)).astype(f8))
    # x32 on Wv likewise; divided back out in the host's final division.
    wvh, wvl = _split8(32.0 * np.asarray(W_v, dtype=np.float32))
    wvh, wvl = _pack(wvh), _pack(wvl)

    cols = [_parity_cols(0), _parity_cols(1)]
    per_batch = []
    for b in range(B):
        xqT = np.ascontiguousarray(X[b].T)
        xqh, xql = _split8(xqT)
        pb = []
        for h in range(2):
            xkh, xkl = xqh[:, cols[h]], xql[:, cols[h]]
            pb.append((_pack(xkh), _pack(xkl)))
        per_batch.append((_pack(xqh), _pack(xql), pb))
    bands = []
    for h in range(2):
        x = np.arange(256)[None, :]
        p = np.arange(P)[:, None]
        bands.append(np.ascontiguousarray((x >= p + P * h).astype(np.float32)))

    in_maps = []
    for c in range(N_CORES):
        b, h = divmod(c, 2)
        xqh, xql, pb = per_batch[b]
        xkh, xkl = pb[h]
        in_maps.append(
            {
                "xk1h": np.ascontiguousarray(xkh[:, :, 0:512]),
                "xk1l": np.ascontiguousarray(xkl[:, :, 0:512]),
                "xk2h": np.ascontiguousarray(xkh[:, :, 512:1024]),
                "xk2l": np.ascontiguousarray(xkl[:, :, 512:1024]),
                "xqh": xqh,
                "xql": xql,
                **{
                    f"wm{e}": np.ascontiguousarray(wm[:, :, e * P : (e + 1) * P])
                    for e in range(8)
                },
                "wvh": wvh,
                "wvl": wvl,
                "band": bands[h],
            }
        )

    nc = _get_nc()
    res = None
    for attempt in range(3):
        try:
            res = run_bass_kernel_spmd(
                nc, in_maps, core_ids=list(range(N_CORES)), **(_run_kwargs or {})
            )
            # materialize now: device failures surface lazily at fetch time,
            # and they must land inside this retry loop
            res.results = [
                {k: np.asarray(v) for k, v in r.items()} for r in res.results
            ]
            break
        except Exception:
            # transient device wedges (NRT_EXEC_UNIT_UNRECOVERABLE) usually
            # clear on retry; drop the poisoned PJRT client first
            if attempt == 2:
                raise
            print(f"kernel: device run failed (attempt {attempt}), retrying",
                  file=sys.stderr)
            import time

            try:
                import jax
                import jax.extend.backend

                jax.clear_caches()
                jax.extend.backend.clear_backends()
            except Exception:
                pass
            time.sleep(3)
    if _results_out is not None:
        _results_out.append(res)

    out = np.empty((B, S, D), dtype=np.float32)
    for b in range(B):
        r0, r1 = res.results[2 * b], res.results[2 * b + 1]
        num = r0["num"].astype(np.float32) + r1["num"].astype(np.float32)
        den = (r0["den"] + r1["den"]).T.reshape(S, 1)  # [p, g] -> row g*128+p
        # numerator carries the x32 Wv host scale; denominator does not
        out[b] = num / (32.0 * den)
    return out
